# revision 17
# baseline (speedup 1.0000x reference)
"""DistMult decoder on 8 Trainium2 NeuronCores.

reference: out[k, i, j] = sigmoid( sum_d x_i[i, d] * relations[k, d] * x_j[j, d] )
shapes: x_i [4096, 128] f32, x_j [4096, 128] f32, relations [8, 128] f32
output: [8, 4096, 4096] f32 (512 MiB)

Sharding: rows of x_i (N_i axis) split across the 8 cores (512 rows each);
x_j and relations replicated. Each core computes its [8, 512, 4096] slab.

The tolerance (rel err < 2e-2) allows storing the sigmoid output as fp16
(quantization ~2e-4) and computing the scores from bf16-rounded operands
(single-pass error ~1.1e-2, validated bit-exactly against the HW), so:

  - stores are fp16: 32 MiB per core (~94 us of HBM time, was the fp32
    store wall of ~190 us that bounded the previous version)
  - matmul is a single bf16 pass (~58 us of PE time; the hi/lo 3-pass
    split is unnecessary at this tolerance)
  - the bottleneck is now sigmoid: the ACT engine runs 1 elem/cycle/lane
    @1.2 GHz = ~1.85 us per [128, 2048] PSUM tile, 64 tiles = ~118 us,
    and everything else (PE, DVE weight prep, both DMA rings) hides
    underneath it.

Rejected alternatives, for the record: a custom fused DVE sigmoid
(2 ops/tile) would halve the sigmoid wall, but this container's walrus
rejects the CUSTOM_DVE_ANT ISA encoding ("ISA wrong length"); a stock-op
DVE polynomial offload (mode "hv", deg-13 clamped odd Horner chain) is
net NEGATIVE because scalar_tensor_tensor runs at half rate and the DVE
SBUF traffic slows every concurrent ACTIVATE by ~8%; u8 output quanting
is dead because ACT's u8 cast rounds sigmoid to {0,1} (no output scale)
and routing sigmoid off ACT doesn't pay (above).

Per-core pipeline:
  - inputs arrive pre-transposed ([D, N] layout, host-side np transpose) so
    the contraction dim D=128 is the SBUF partition dim for both matmul
    operands; x_j^T arrives pre-rounded to bf16 from the host.
  - per relation k: scale x_i^T columns by r_k straight to bf16
    (tensor_scalar with casting output), prefetched one k ahead.
  - matmul per 512-wide PSUM bank chunk, psum tiles [128, 2048] x 2
  - sigmoid on ACT (or DVE), fp16 into SBUF
  - 1 MiB DMA per [128, 4096] fp16 result block, alternating between the
    SP hardware DGE ring and the GpSimd software DGE ring
  - host upcasts the returned fp16 slabs to f32
"""

import os

import numpy as np

import concourse.bass as bass
import concourse.mybir as mybir
from concourse import tile
from concourse.bass_utils import run_bass_kernel_spmd

N_I, N_J, D, K = 4096, 4096, 128, 8
N_CORES = 8
SHARD = N_I // N_CORES  # 512
P = 128
HALF = N_J // 2  # 2048
F32 = mybir.dt.float32
F16 = mybir.dt.float16
BF16 = mybir.dt.bfloat16
ALU = mybir.AluOpType

# "hv" = 1-pass bf16 matmul + DVE sigmoid offload; "h1" = ACT-only sigmoid.
MODE = os.environ.get("DISTMULT_MODE", "h1")

# sigma(x) ~= relu(min(x*(c0 + P(w)) + 0.5, 1)), w = lam*x^2,
# P = ((((w+s1)w+s2)w+s3)w+s4)w+s5)w via (h+s)*w Horner steps.
# Minimax-fitted over |x| <= 18.6 (scores reach +-17.8); max err 1.36e-3.
SIG_LAM = 0.0230786155
SIG_S = (-3.79432826, 5.89380247, -4.90242032, 2.4424166, -0.812018308)
SIG_C0 = 0.248215618
# regular-tile indices (0..61) evaluated on the DVE instead of ACT
DVE_TILES = frozenset((5, 17, 29, 41, 53))


def _split_ctrl_waits(nc, maxw=1):
    """walrus in this container accepts only one sync-wait on several
    instruction structs (Drain/TPB_CTRL, tensor_scalar/S3D3_TS, ...); move
    excess waits onto same-engine NOPs placed immediately before. Engines
    consume their queues in order, so waiting on A (NOP) then B (inst) is
    equivalent to the inst waiting on both."""
    for f in nc.m.functions:
        for bb in f.blocks:
            newinsts = []
            for i in bb.instructions:
                si = i.sync_info
                if si is not None and len(si.on_wait) > maxw:
                    waits = list(si.on_wait)
                    extra, keep = waits[:-maxw], waits[-maxw:]
                    for idx in range(0, len(extra), maxw):
                        nop = mybir.InstNoOp(name=f"{i.name}-ws{idx}", ins=[], outs=[])
                        nop.engine = i.engine
                        nop.sync_info = mybir.SyncInfo(
                            on_wait=extra[idx : idx + maxw], on_update=[]
                        )
                        nc.register_instruction(nop)
                        newinsts.append(nop)
                    si.on_wait = keep
                newinsts.append(i)
            bb.instructions[:] = newinsts


def build(mode=MODE):
    nc = bass.Bass()
    x_iT = nc.dram_tensor("x_iT", [D, SHARD], F32, kind="ExternalInput")
    relT = nc.dram_tensor("relT", [D, K], F32, kind="ExternalInput")
    x_i0T = nc.dram_tensor("x_i0T", [D, P], F32, kind="ExternalInput")
    x_jT_hi = nc.dram_tensor("x_jT_hi", [D, N_J], BF16, kind="ExternalInput")
    out = nc.dram_tensor("out", [K, SHARD, N_J], F16, kind="ExternalOutput")

    with tile.TileContext(nc) as tc:
        with (
            tc.tile_pool(name="const", bufs=1) as const,
            tc.tile_pool(name="w", bufs=2) as wpool,
            tc.tile_pool(name="psum", bufs=2, space=bass.MemorySpace.PSUM) as psum,
            tc.tile_pool(name="ob", bufs=6) as obuf,
            tc.tile_pool(name="obs", bufs=6) as obuf_small,
            tc.tile_pool(name="vch", bufs=2 if mode == "hv" else 1) as vpool,
        ):
            # input loads. sync ring carries the small early blockers in
            # dependency order; the first big rhs chunk rides the scalar
            # ring in parallel; x_iT (not needed until k0/m1) goes via the
            # GpSimd SWDGE ring to keep the HWDGE rings short.
            xi0 = const.tile([P, P], F32, tag="xi0")
            nc.sync.dma_start(xi0[:], x_i0T[:])
            rel = const.tile([P, K], F32, tag="rel")
            nc.sync.dma_start(rel[:], relT[:])
            xjh0a = const.tile([P, 512], BF16, tag="xjh0a")
            nc.sync.dma_start(xjh0a[:], x_jT_hi[:, 0:512])
            rh = [None] * 4
            for s, t_ in ((0, "scalar"), (1, "sync"), (2, "sync"), (3, "sync")):
                rht = const.tile([P, 1024], BF16, tag=f"xjh{s}")
                rh[s] = rht
                eng = nc.scalar if t_ == "scalar" else nc.sync
                eng.dma_start(rht[:], x_jT_hi[:, s * 1024 : (s + 1) * 1024])
            # k0/m1 weight chunk: 64 KB of x_iT (cols 128:256) lands well
            # before the full x_iT, unblocking the second row block early
            xic1 = const.tile([P, P], F32, tag="xic1")
            nc.scalar.dma_start(xic1[:], x_iT[:, P : 2 * P])
            xiT = const.tile([P, SHARD], F32, tag="xiT")
            nc.scalar.dma_start(xiT[:], x_iT[:])

            # warm up the sigmoid spline tables (~2.7us) under the input DMAs
            scratch = const.tile([P, 1], F32, tag="scratch")
            nc.gpsimd.memset(scratch[:], 0.0)
            nc.scalar.activation(
                scratch[:], scratch[:], mybir.ActivationFunctionType.Sigmoid
            )

            # warm up the PE clock (HAM un-throttles after ~3.4us of sustained
            # matmul activity) with dummy matmuls while the inputs stream in
            wmup = const.tile([P, 512], BF16, tag="wmup")
            nc.gpsimd.memset(wmup[:], 0.0)
            wps = psum.tile([P, HALF], F32, tag="ps")
            for r in range(8):
                nc.tensor.matmul(
                    wps[:, (r % 4) * 512 : (r % 4 + 1) * 512],
                    wmup[:, 0:P],
                    wmup[:],
                    start=True,
                    stop=True,
                )
            # reader keeps the warmup matmuls live through any dead-code pass
            nc.scalar.activation(
                scratch[:], wps[:, 0:1], mybir.ActivationFunctionType.Sigmoid
            )

            # fast-path k=0 weights for the first 128-row block, fed from the
            # tiny xi0 load so the first matmuls start early
            wk0_hi = const.tile([P, P], BF16, tag="wk0_hi")
            nc.vector.tensor_scalar_mul(wk0_hi[:], xi0[:], rel[:, 0:1])
            wk0b1 = const.tile([P, P], BF16, tag="wk0b1")
            nc.vector.tensor_scalar_mul(wk0b1[:], xic1[:], rel[:, 0:1])

            wks = {}

            def prep_wk(k):
                if k >= K or k in wks:
                    return
                t = wpool.tile([P, SHARD], BF16, tag="wk_hi")
                nc.vector.tensor_scalar_mul(t[:], xiT[:], rel[:, k : k + 1])
                wks[k] = t

            def dve_sigmoid(ps, obslice):
                """Clamped deg-13 odd-poly sigmoid on the DVE; first op
                copies the scores out of PSUM so the bank frees early."""
                xs = vpool.tile([P, HALF], F32, tag="xs")
                nc.vector.tensor_copy(xs[:], ps[:])
                w = vpool.tile([P, HALF], F32, tag="w")
                nc.vector.scalar_tensor_tensor(
                    w[:], xs[:], SIG_LAM, xs[:], ALU.mult, ALU.mult
                )
                ha = vpool.tile([P, HALF], F32, tag="ha")
                hb = vpool.tile([P, HALF], F32, tag="hb")
                cur, nxt = w, ha
                for s in SIG_S:
                    nc.vector.scalar_tensor_tensor(
                        nxt[:], cur[:], float(s), w[:], ALU.add, ALU.mult
                    )
                    cur, nxt = nxt, (hb if nxt is ha else ha)
                # t = (h + c0) * x ; sc = min(t + 0.5, 1) ; out = max(sc, 0)
                nc.vector.scalar_tensor_tensor(
                    nxt[:], cur[:], SIG_C0, xs[:], ALU.add, ALU.mult
                )
                sc = ha if nxt is hb else hb
                nc.vector.tensor_scalar(
                    sc[:], nxt[:], 0.5, 1.0, ALU.add, ALU.min
                )
                nc.vector.tensor_scalar_max(obslice, sc[:], 0.0)

            chunk = 0
            tix = 0  # regular-tile counter
            for k in range(K):
                prep_wk(k)
                for m in range(SHARD // P):  # 4 row blocks of 128
                    mc = slice(m * P, (m + 1) * P)
                    if k == 0 and m == 0:
                        # extra-fine first block: a leading 512-wide sub-chunk
                        # fed from the tiny duplicated loads so the store
                        # stream starts while the PE is still ramping
                        subs = [
                            (0, 512, xjh0a, 0),
                            (512, 512, rh[0], 512),
                            (1024, 1024, rh[1], 0),
                            (2048, 1024, rh[2], 0),
                            (3072, 1024, rh[3], 0),
                        ]
                        for c0, w_, th, off in subs:
                            psq = psum.tile([P, w_], F32, tag="ps")
                            for n2 in range(w_ // 512):
                                nc.tensor.matmul(
                                    psq[:, n2 * 512 : (n2 + 1) * 512],
                                    wk0_hi[:],
                                    th[:, off + n2 * 512 : off + (n2 + 1) * 512],
                                    start=True,
                                    stop=True,
                                )
                            obq = obuf_small.tile([P, w_], F16, tag="obs")
                            nc.scalar.activation(
                                obq[:], psq[:], mybir.ActivationFunctionType.Sigmoid
                            )
                            eng = nc.sync if chunk % 2 == 0 else nc.gpsimd
                            eng.dma_start(out[0, 0:P, c0 : c0 + w_], obq[:])
                            chunk += 1
                        continue
                    if m == 1:
                        prep_wk(k + 1)
                    wk_slice = wk0b1[:] if (k == 0 and m == 1) else wks[k][:, mc]
                    fine = k == K - 1 and m == SHARD // P - 1
                    ob = None if fine else obuf.tile([P, N_J], F16, tag="ob")
                    for h in range(2):  # two 2048-wide PSUM tiles per block
                        ps = psum.tile([P, HALF], F32, tag="ps")
                        for n4 in range(4):  # one 512-wide matmul per bank
                            gc = h * HALF + n4 * 512
                            nc.tensor.matmul(
                                ps[:, n4 * 512 : (n4 + 1) * 512],
                                wk_slice,
                                rh[gc // 1024][:, gc % 1024 : gc % 1024 + 512],
                                start=True,
                                stop=True,
                            )
                        if fine:
                            if h == 0:
                                obh = obuf_small.tile([P, HALF], F16, tag="obs")
                                nc.scalar.activation(
                                    obh[:], ps[:],
                                    mybir.ActivationFunctionType.Sigmoid,
                                )
                                nc.sync.dma_start(out[k, mc, 0:HALF], obh[:])
                            else:
                                # taper the very last stores (1024+512+512) so
                                # the kernel-final DMA is small before drain
                                for o0, w_, eng in (
                                    (0, 1024, nc.gpsimd),
                                    (1024, 512, nc.scalar),
                                    (1536, 512, nc.sync),
                                ):
                                    obt = obuf_small.tile([P, w_], F16, tag="obs")
                                    nc.scalar.activation(
                                        obt[:], ps[:, o0 : o0 + w_],
                                        mybir.ActivationFunctionType.Sigmoid,
                                    )
                                    eng.dma_start(
                                        out[k, mc, HALF + o0 : HALF + o0 + w_],
                                        obt[:],
                                    )
                            chunk += 1
                        else:
                            obslice = ob[:, h * HALF : (h + 1) * HALF]
                            if mode == "hv" and tix in DVE_TILES:
                                dve_sigmoid(ps, obslice)
                            else:
                                nc.scalar.activation(
                                    obslice, ps[:],
                                    mybir.ActivationFunctionType.Sigmoid,
                                )
                            tix += 1
                    if not fine:
                        eng = nc.sync if chunk % 5 < 3 else nc.gpsimd
                        eng.dma_start(out[k, mc, :], ob[:])
                        chunk += 1

    _split_ctrl_waits(nc)
    return nc


_cache = {}


def kernel(x_i, x_j, relations):
    x_i = np.asarray(x_i, dtype=np.float32)
    x_j = np.asarray(x_j, dtype=np.float32)
    relations = np.asarray(relations, dtype=np.float32)
    assert x_i.shape == (N_I, D) and x_j.shape == (N_J, D)
    assert relations.shape == (K, D)

    if MODE not in _cache:
        _cache[MODE] = build(MODE)
    nc = _cache[MODE]

    import ml_dtypes

    x_jT = np.ascontiguousarray(x_j.T)
    relT = np.ascontiguousarray(relations.T)
    common = {"relT": relT, "x_jT_hi": x_jT.astype(ml_dtypes.bfloat16)}

    in_maps = []
    for c in range(N_CORES):
        shard = np.ascontiguousarray(x_i[c * SHARD : (c + 1) * SHARD, :].T)
        in_maps.append(
            {"x_iT": shard, "x_i0T": np.ascontiguousarray(shard[:, 0:P]), **common}
        )

    trace = bool(int(os.environ.get("DISTMULT_TRACE", "0")))
    res = run_bass_kernel_spmd(nc, in_maps, list(range(N_CORES)), trace=trace)
    if trace:
        kernel.last_exec_time_ns = res.exec_time_ns
        kernel.last_results = res
    halves = [res.results[c]["out"] for c in range(N_CORES)]
    return np.concatenate(halves, axis=1).astype(np.float32)


# revision 18
# speedup vs baseline: 1.0149x; 1.0149x over previous
"""DistMult decoder on 8 Trainium2 NeuronCores.

reference: out[k, i, j] = sigmoid( sum_d x_i[i, d] * relations[k, d] * x_j[j, d] )
shapes: x_i [4096, 128] f32, x_j [4096, 128] f32, relations [8, 128] f32
output: [8, 4096, 4096] f32 (512 MiB)

Sharding: rows of x_i (N_i axis) split across the 8 cores (512 rows each);
x_j and relations replicated. Each core computes its [8, 512, 4096] slab.

The tolerance (rel err < 2e-2) allows storing the sigmoid output as fp16
(quantization ~2e-4) and computing the scores from bf16-rounded operands
(single-pass error ~1.1e-2, validated bit-exactly against the HW), so:

  - stores are fp16: 32 MiB per core (~94 us of HBM time, was the fp32
    store wall of ~190 us that bounded the previous version)
  - matmul is a single bf16 pass (~58 us of PE time; the hi/lo 3-pass
    split is unnecessary at this tolerance)
  - the bottleneck is now sigmoid: the ACT engine runs 1 elem/cycle/lane
    @1.2 GHz = ~1.85 us per [128, 2048] PSUM tile, 64 tiles = ~118 us,
    and everything else (PE, DVE weight prep, both DMA rings) hides
    underneath it.

Rejected alternatives, for the record: a custom fused DVE sigmoid
(2 ops/tile) would halve the sigmoid wall, but this container's walrus
rejects the CUSTOM_DVE_ANT ISA encoding ("ISA wrong length"); a stock-op
DVE polynomial offload (mode "hv", deg-13 clamped odd Horner chain) is
net NEGATIVE because scalar_tensor_tensor runs at half rate and the DVE
SBUF traffic slows every concurrent ACTIVATE by ~8%; u8 output quanting
is dead because ACT's u8 cast rounds sigmoid to {0,1} (no output scale)
and routing sigmoid off ACT doesn't pay (above).

Per-core pipeline:
  - inputs arrive pre-transposed ([D, N] layout, host-side np transpose) so
    the contraction dim D=128 is the SBUF partition dim for both matmul
    operands; x_j^T arrives pre-rounded to bf16 from the host.
  - per relation k: scale x_i^T columns by r_k straight to bf16
    (tensor_scalar with casting output), prefetched one k ahead.
  - matmul per 512-wide PSUM bank chunk, psum tiles [128, 2048] x 2
  - sigmoid on ACT (or DVE), fp16 into SBUF
  - 1 MiB DMA per [128, 4096] fp16 result block, alternating between the
    SP hardware DGE ring and the GpSimd software DGE ring
  - host upcasts the returned fp16 slabs to f32
"""

import os

import numpy as np

import concourse.bass as bass
import concourse.mybir as mybir
from concourse import tile
from concourse.bass_utils import run_bass_kernel_spmd

N_I, N_J, D, K = 4096, 4096, 128, 8
N_CORES = 8
SHARD = N_I // N_CORES  # 512
P = 128
HALF = N_J // 2  # 2048
F32 = mybir.dt.float32
F16 = mybir.dt.float16
BF16 = mybir.dt.bfloat16
ALU = mybir.AluOpType

# "hv" = 1-pass bf16 matmul + DVE sigmoid offload; "h1" = ACT-only sigmoid.
MODE = os.environ.get("DISTMULT_MODE", "h1")

# sigma(x) ~= relu(min(x*(c0 + P(w)) + 0.5, 1)), w = lam*x^2,
# P = ((((w+s1)w+s2)w+s3)w+s4)w+s5)w via (h+s)*w Horner steps.
# Minimax-fitted over |x| <= 18.6 (scores reach +-17.8); max err 1.36e-3.
SIG_LAM = 0.0230786155
SIG_S = (-3.79432826, 5.89380247, -4.90242032, 2.4424166, -0.812018308)
SIG_C0 = 0.248215618
# regular-tile indices (0..61) evaluated on the DVE instead of ACT
DVE_TILES = frozenset((5, 17, 29, 41, 53))


def _split_ctrl_waits(nc, maxw=1):
    """walrus in this container accepts only one sync-wait on several
    instruction structs (Drain/TPB_CTRL, tensor_scalar/S3D3_TS, ...); move
    excess waits onto same-engine NOPs placed immediately before. Engines
    consume their queues in order, so waiting on A (NOP) then B (inst) is
    equivalent to the inst waiting on both."""
    for f in nc.m.functions:
        for bb in f.blocks:
            newinsts = []
            for i in bb.instructions:
                si = i.sync_info
                if si is not None and len(si.on_wait) > maxw:
                    waits = list(si.on_wait)
                    extra, keep = waits[:-maxw], waits[-maxw:]
                    for idx in range(0, len(extra), maxw):
                        nop = mybir.InstNoOp(name=f"{i.name}-ws{idx}", ins=[], outs=[])
                        nop.engine = i.engine
                        nop.sync_info = mybir.SyncInfo(
                            on_wait=extra[idx : idx + maxw], on_update=[]
                        )
                        nc.register_instruction(nop)
                        newinsts.append(nop)
                    si.on_wait = keep
                newinsts.append(i)
            bb.instructions[:] = newinsts


def build(mode=MODE):
    nc = bass.Bass()
    x_iT = nc.dram_tensor("x_iT", [D, SHARD], F32, kind="ExternalInput")
    relT = nc.dram_tensor("relT", [D, K], F32, kind="ExternalInput")
    x_i0T = nc.dram_tensor("x_i0T", [D, P], F32, kind="ExternalInput")
    x_jT_hi = nc.dram_tensor("x_jT_hi", [D, N_J], BF16, kind="ExternalInput")
    out = nc.dram_tensor("out", [K, SHARD, N_J], F16, kind="ExternalOutput")

    with tile.TileContext(nc) as tc:
        with (
            tc.tile_pool(name="const", bufs=1) as const,
            tc.tile_pool(name="w", bufs=2) as wpool,
            tc.tile_pool(name="psum", bufs=2, space=bass.MemorySpace.PSUM) as psum,
            tc.tile_pool(name="ob", bufs=4) as obuf,
            tc.tile_pool(name="obs", bufs=6) as obuf_small,
            tc.tile_pool(name="vch", bufs=2 if mode == "hv" else 1) as vpool,
        ):
            # input loads. sync ring carries the small early blockers in
            # dependency order; the first big rhs chunk rides the scalar
            # ring in parallel; x_iT (not needed until k0/m1) goes via the
            # GpSimd SWDGE ring to keep the HWDGE rings short.
            xi0 = const.tile([P, P], F32, tag="xi0")
            nc.sync.dma_start(xi0[:], x_i0T[:])
            rel = const.tile([P, K], F32, tag="rel")
            nc.sync.dma_start(rel[:], relT[:])
            xjh0a = const.tile([P, 512], BF16, tag="xjh0a")
            nc.sync.dma_start(xjh0a[:], x_jT_hi[:, 0:512])
            rh = [None] * 4
            for s, t_ in ((0, "scalar"), (1, "sync"), (2, "sync"), (3, "sync")):
                rht = const.tile([P, 1024], BF16, tag=f"xjh{s}")
                rh[s] = rht
                eng = nc.scalar if t_ == "scalar" else nc.sync
                eng.dma_start(rht[:], x_jT_hi[:, s * 1024 : (s + 1) * 1024])
            # k0/m1 weight chunk: 64 KB of x_iT (cols 128:256) lands well
            # before the full x_iT, unblocking the second row block early
            xic1 = const.tile([P, P], F32, tag="xic1")
            nc.scalar.dma_start(xic1[:], x_iT[:, P : 2 * P])
            xiT = const.tile([P, SHARD], F32, tag="xiT")
            nc.scalar.dma_start(xiT[:], x_iT[:])

            # warm up the sigmoid spline tables (~2.7us) under the input DMAs
            scratch = const.tile([P, 1], F32, tag="scratch")
            nc.gpsimd.memset(scratch[:], 0.0)
            nc.scalar.activation(
                scratch[:], scratch[:], mybir.ActivationFunctionType.Sigmoid
            )

            # warm up the PE clock (HAM un-throttles after ~3.4us of sustained
            # matmul activity) with dummy matmuls while the inputs stream in
            wmup = const.tile([P, 512], BF16, tag="wmup")
            nc.gpsimd.memset(wmup[:], 0.0)
            wps = psum.tile([P, HALF], F32, tag="ps")
            for r in range(8):
                nc.tensor.matmul(
                    wps[:, (r % 4) * 512 : (r % 4 + 1) * 512],
                    wmup[:, 0:P],
                    wmup[:],
                    start=True,
                    stop=True,
                )
            # reader keeps the warmup matmuls live through any dead-code pass
            nc.scalar.activation(
                scratch[:], wps[:, 0:1], mybir.ActivationFunctionType.Sigmoid
            )

            # fast-path k=0 weights for the first 128-row block, fed from the
            # tiny xi0 load so the first matmuls start early
            wk0_hi = const.tile([P, P], BF16, tag="wk0_hi")
            nc.vector.tensor_scalar_mul(wk0_hi[:], xi0[:], rel[:, 0:1])
            wk0b1 = const.tile([P, P], BF16, tag="wk0b1")
            nc.vector.tensor_scalar_mul(wk0b1[:], xic1[:], rel[:, 0:1])

            wks = {}

            def prep_wk(k):
                if k >= K or k in wks:
                    return
                t = wpool.tile([P, SHARD], BF16, tag="wk_hi")
                nc.vector.tensor_scalar_mul(t[:], xiT[:], rel[:, k : k + 1])
                wks[k] = t

            def dve_sigmoid(ps, obslice):
                """Clamped deg-13 odd-poly sigmoid on the DVE; first op
                copies the scores out of PSUM so the bank frees early."""
                xs = vpool.tile([P, HALF], F32, tag="xs")
                nc.vector.tensor_copy(xs[:], ps[:])
                w = vpool.tile([P, HALF], F32, tag="w")
                nc.vector.scalar_tensor_tensor(
                    w[:], xs[:], SIG_LAM, xs[:], ALU.mult, ALU.mult
                )
                ha = vpool.tile([P, HALF], F32, tag="ha")
                hb = vpool.tile([P, HALF], F32, tag="hb")
                cur, nxt = w, ha
                for s in SIG_S:
                    nc.vector.scalar_tensor_tensor(
                        nxt[:], cur[:], float(s), w[:], ALU.add, ALU.mult
                    )
                    cur, nxt = nxt, (hb if nxt is ha else ha)
                # t = (h + c0) * x ; sc = min(t + 0.5, 1) ; out = max(sc, 0)
                nc.vector.scalar_tensor_tensor(
                    nxt[:], cur[:], SIG_C0, xs[:], ALU.add, ALU.mult
                )
                sc = ha if nxt is hb else hb
                nc.vector.tensor_scalar(
                    sc[:], nxt[:], 0.5, 1.0, ALU.add, ALU.min
                )
                nc.vector.tensor_scalar_max(obslice, sc[:], 0.0)

            chunk = 0
            tix = 0  # regular-tile counter
            for k in range(K):
                prep_wk(k)
                for m in range(SHARD // P):  # 4 row blocks of 128
                    mc = slice(m * P, (m + 1) * P)
                    if k == 0 and m == 0:
                        # extra-fine first block: a leading 512-wide sub-chunk
                        # fed from the tiny duplicated loads so the store
                        # stream starts while the PE is still ramping
                        subs = [
                            (0, 512, xjh0a, 0),
                            (512, 512, rh[0], 512),
                            (1024, 1024, rh[1], 0),
                            (2048, 1024, rh[2], 0),
                            (3072, 1024, rh[3], 0),
                        ]
                        for c0, w_, th, off in subs:
                            psq = psum.tile([P, w_], F32, tag="ps")
                            for n2 in range(w_ // 512):
                                nc.tensor.matmul(
                                    psq[:, n2 * 512 : (n2 + 1) * 512],
                                    wk0_hi[:],
                                    th[:, off + n2 * 512 : off + (n2 + 1) * 512],
                                    start=True,
                                    stop=True,
                                )
                            obq = obuf_small.tile([P, w_], F16, tag="obs")
                            nc.scalar.activation(
                                obq[:], psq[:], mybir.ActivationFunctionType.Sigmoid
                            )
                            eng = nc.sync if chunk % 2 == 0 else nc.gpsimd
                            eng.dma_start(out[0, 0:P, c0 : c0 + w_], obq[:])
                            chunk += 1
                        continue
                    if m == 1:
                        prep_wk(k + 1)
                    wk_slice = wk0b1[:] if (k == 0 and m == 1) else wks[k][:, mc]
                    fine = k == K - 1 and m == SHARD // P - 1
                    ob = None if fine else obuf.tile([P, N_J], F16, tag="ob")
                    for h in range(2):  # two 2048-wide PSUM tiles per block
                        ps = psum.tile([P, HALF], F32, tag="ps")
                        for n4 in range(4):  # one 512-wide matmul per bank
                            gc = h * HALF + n4 * 512
                            nc.tensor.matmul(
                                ps[:, n4 * 512 : (n4 + 1) * 512],
                                wk_slice,
                                rh[gc // 1024][:, gc % 1024 : gc % 1024 + 512],
                                start=True,
                                stop=True,
                            )
                        if fine:
                            if h == 0:
                                obh = obuf_small.tile([P, HALF], F16, tag="obs")
                                nc.scalar.activation(
                                    obh[:], ps[:],
                                    mybir.ActivationFunctionType.Sigmoid,
                                )
                                nc.sync.dma_start(out[k, mc, 0:HALF], obh[:])
                            else:
                                # taper the very last stores (1024+512+512) so
                                # the kernel-final DMA is small before drain
                                for o0, w_, eng in (
                                    (0, 1024, nc.gpsimd),
                                    (1024, 512, nc.scalar),
                                    (1536, 512, nc.sync),
                                ):
                                    obt = obuf_small.tile([P, w_], F16, tag="obs")
                                    nc.scalar.activation(
                                        obt[:], ps[:, o0 : o0 + w_],
                                        mybir.ActivationFunctionType.Sigmoid,
                                    )
                                    eng.dma_start(
                                        out[k, mc, HALF + o0 : HALF + o0 + w_],
                                        obt[:],
                                    )
                            chunk += 1
                        else:
                            obslice = ob[:, h * HALF : (h + 1) * HALF]
                            if mode == "hv" and tix in DVE_TILES:
                                dve_sigmoid(ps, obslice)
                            else:
                                nc.scalar.activation(
                                    obslice, ps[:],
                                    mybir.ActivationFunctionType.Sigmoid,
                                )
                            tix += 1
                    if not fine:
                        eng = nc.sync if chunk % 2 == 0 else nc.gpsimd
                        eng.dma_start(out[k, mc, :], ob[:])
                        chunk += 1

    _split_ctrl_waits(nc)
    return nc


_cache = {}


def kernel(x_i, x_j, relations):
    x_i = np.asarray(x_i, dtype=np.float32)
    x_j = np.asarray(x_j, dtype=np.float32)
    relations = np.asarray(relations, dtype=np.float32)
    assert x_i.shape == (N_I, D) and x_j.shape == (N_J, D)
    assert relations.shape == (K, D)

    if MODE not in _cache:
        _cache[MODE] = build(MODE)
    nc = _cache[MODE]

    import ml_dtypes

    x_jT = np.ascontiguousarray(x_j.T)
    relT = np.ascontiguousarray(relations.T)
    common = {"relT": relT, "x_jT_hi": x_jT.astype(ml_dtypes.bfloat16)}

    in_maps = []
    for c in range(N_CORES):
        shard = np.ascontiguousarray(x_i[c * SHARD : (c + 1) * SHARD, :].T)
        in_maps.append(
            {"x_iT": shard, "x_i0T": np.ascontiguousarray(shard[:, 0:P]), **common}
        )

    trace = bool(int(os.environ.get("DISTMULT_TRACE", "0")))
    res = run_bass_kernel_spmd(nc, in_maps, list(range(N_CORES)), trace=trace)
    if trace:
        kernel.last_exec_time_ns = res.exec_time_ns
        kernel.last_results = res
    halves = [res.results[c]["out"] for c in range(N_CORES)]
    return np.concatenate(halves, axis=1).astype(np.float32)


# revision 19
# speedup vs baseline: 1.0175x; 1.0026x over previous
"""DistMult decoder on 8 Trainium2 NeuronCores.

reference: out[k, i, j] = sigmoid( sum_d x_i[i, d] * relations[k, d] * x_j[j, d] )
shapes: x_i [4096, 128] f32, x_j [4096, 128] f32, relations [8, 128] f32
output: [8, 4096, 4096] f32 (512 MiB)

Sharding: rows of x_i (N_i axis) split across the 8 cores (512 rows each);
x_j and relations replicated. Each core computes its [8, 512, 4096] slab.

The tolerance (rel err < 2e-2) allows storing the sigmoid output as fp16
(quantization ~2e-4) and computing the scores from bf16-rounded operands
(single-pass error ~1.1e-2, validated bit-exactly against the HW), so:

  - stores are fp16: 32 MiB per core (~94 us of HBM time, was the fp32
    store wall of ~190 us that bounded the previous version)
  - matmul is a single bf16 pass (~58 us of PE time; the hi/lo 3-pass
    split is unnecessary at this tolerance)
  - the bottleneck is now sigmoid: the ACT engine runs 1 elem/cycle/lane
    @1.2 GHz = ~1.85 us per [128, 2048] PSUM tile, 64 tiles = ~118 us,
    and everything else (PE, DVE weight prep, both DMA rings) hides
    underneath it.

Rejected alternatives, for the record: a custom fused DVE sigmoid
(2 ops/tile) would halve the sigmoid wall, but this container's walrus
rejects the CUSTOM_DVE_ANT ISA encoding ("ISA wrong length"); a stock-op
DVE polynomial offload (mode "hv", deg-13 clamped odd Horner chain) is
net NEGATIVE because scalar_tensor_tensor runs at half rate and the DVE
SBUF traffic slows every concurrent ACTIVATE by ~8%; u8 output quanting
is dead because ACT's u8 cast rounds sigmoid to {0,1} (no output scale)
and routing sigmoid off ACT doesn't pay (above).

Per-core pipeline:
  - inputs arrive pre-transposed ([D, N] layout, host-side np transpose) so
    the contraction dim D=128 is the SBUF partition dim for both matmul
    operands; x_j^T arrives pre-rounded to bf16 from the host.
  - per relation k: scale x_i^T columns by r_k straight to bf16
    (tensor_scalar with casting output), prefetched one k ahead.
  - matmul per 512-wide PSUM bank chunk, psum tiles [128, 2048] x 2
  - sigmoid on ACT (or DVE), fp16 into SBUF
  - 1 MiB DMA per [128, 4096] fp16 result block, alternating between the
    SP hardware DGE ring and the GpSimd software DGE ring
  - host upcasts the returned fp16 slabs to f32
"""

import os

import numpy as np

import concourse.bass as bass
import concourse.mybir as mybir
from concourse import tile
from concourse.bass_utils import run_bass_kernel_spmd

N_I, N_J, D, K = 4096, 4096, 128, 8
N_CORES = 8
SHARD = N_I // N_CORES  # 512
P = 128
HALF = N_J // 2  # 2048
F32 = mybir.dt.float32
F16 = mybir.dt.float16
BF16 = mybir.dt.bfloat16
ALU = mybir.AluOpType

# "hv" = 1-pass bf16 matmul + DVE sigmoid offload; "h1" = ACT-only sigmoid.
MODE = os.environ.get("DISTMULT_MODE", "h1")

# sigma(x) ~= relu(min(x*(c0 + P(w)) + 0.5, 1)), w = lam*x^2,
# P = ((((w+s1)w+s2)w+s3)w+s4)w+s5)w via (h+s)*w Horner steps.
# Minimax-fitted over |x| <= 18.6 (scores reach +-17.8); max err 1.36e-3.
SIG_LAM = 0.0230786155
SIG_S = (-3.79432826, 5.89380247, -4.90242032, 2.4424166, -0.812018308)
SIG_C0 = 0.248215618
# regular-tile indices (0..61) evaluated on the DVE instead of ACT
DVE_TILES = frozenset((5, 17, 29, 41, 53))


def _split_ctrl_waits(nc, maxw=1):
    """walrus in this container accepts only one sync-wait on several
    instruction structs (Drain/TPB_CTRL, tensor_scalar/S3D3_TS, ...); move
    excess waits onto same-engine NOPs placed immediately before. Engines
    consume their queues in order, so waiting on A (NOP) then B (inst) is
    equivalent to the inst waiting on both."""
    for f in nc.m.functions:
        for bb in f.blocks:
            newinsts = []
            for i in bb.instructions:
                si = i.sync_info
                if si is not None and len(si.on_wait) > maxw:
                    waits = list(si.on_wait)
                    extra, keep = waits[:-maxw], waits[-maxw:]
                    for idx in range(0, len(extra), maxw):
                        nop = mybir.InstNoOp(name=f"{i.name}-ws{idx}", ins=[], outs=[])
                        nop.engine = i.engine
                        nop.sync_info = mybir.SyncInfo(
                            on_wait=extra[idx : idx + maxw], on_update=[]
                        )
                        nc.register_instruction(nop)
                        newinsts.append(nop)
                    si.on_wait = keep
                newinsts.append(i)
            bb.instructions[:] = newinsts


def build(mode=MODE):
    nc = bass.Bass()
    x_iT = nc.dram_tensor("x_iT", [D, SHARD], F32, kind="ExternalInput")
    relT = nc.dram_tensor("relT", [D, K], F32, kind="ExternalInput")
    x_i0T = nc.dram_tensor("x_i0T", [D, P], F32, kind="ExternalInput")
    x_jT_hi = nc.dram_tensor("x_jT_hi", [D, N_J], BF16, kind="ExternalInput")
    out = nc.dram_tensor("out", [K, SHARD, N_J], F16, kind="ExternalOutput")

    with tile.TileContext(nc) as tc:
        with (
            tc.tile_pool(name="const", bufs=1) as const,
            tc.tile_pool(name="w", bufs=2) as wpool,
            tc.tile_pool(name="psum", bufs=2, space=bass.MemorySpace.PSUM) as psum,
            tc.tile_pool(name="ob", bufs=4) as obuf,
            tc.tile_pool(name="obs", bufs=6) as obuf_small,
            tc.tile_pool(name="vch", bufs=2 if mode == "hv" else 1) as vpool,
        ):
            # input loads. sync ring carries the small early blockers in
            # dependency order; the first big rhs chunk rides the scalar
            # ring in parallel; x_iT (not needed until k0/m1) goes via the
            # GpSimd SWDGE ring to keep the HWDGE rings short.
            xi0 = const.tile([P, P], F32, tag="xi0")
            nc.sync.dma_start(xi0[:], x_i0T[:])
            rel = const.tile([P, K], F32, tag="rel")
            nc.sync.dma_start(rel[:], relT[:])
            xjh0a = const.tile([P, 512], BF16, tag="xjh0a")
            nc.sync.dma_start(xjh0a[:], x_jT_hi[:, 0:512])
            rh = [None] * 4
            for s, t_ in ((0, "scalar"), (1, "sync"), (2, "sync"), (3, "sync")):
                rht = const.tile([P, 1024], BF16, tag=f"xjh{s}")
                rh[s] = rht
                eng = nc.scalar if t_ == "scalar" else nc.sync
                eng.dma_start(rht[:], x_jT_hi[:, s * 1024 : (s + 1) * 1024])
            # k0/m1 weight chunk: 64 KB of x_iT (cols 128:256) lands well
            # before the full x_iT, unblocking the second row block early
            xic1 = const.tile([P, P], F32, tag="xic1")
            nc.scalar.dma_start(xic1[:], x_iT[:, P : 2 * P])
            xiT = const.tile([P, SHARD], F32, tag="xiT")
            nc.scalar.dma_start(xiT[:], x_iT[:])

            # warm up the sigmoid spline tables (~2.7us) under the input DMAs
            scratch = const.tile([P, 1], F32, tag="scratch")
            nc.gpsimd.memset(scratch[:], 0.0)
            nc.scalar.activation(
                scratch[:], scratch[:], mybir.ActivationFunctionType.Sigmoid
            )

            # warm up the PE clock (HAM un-throttles after ~3.4us of sustained
            # matmul activity) with dummy matmuls while the inputs stream in
            wmup = const.tile([P, 512], BF16, tag="wmup")
            nc.gpsimd.memset(wmup[:], 0.0)
            wps = psum.tile([P, HALF], F32, tag="ps")
            for r in range(8):
                nc.tensor.matmul(
                    wps[:, (r % 4) * 512 : (r % 4 + 1) * 512],
                    wmup[:, 0:P],
                    wmup[:],
                    start=True,
                    stop=True,
                )
            # reader keeps the warmup matmuls live through any dead-code pass
            nc.scalar.activation(
                scratch[:], wps[:, 0:1], mybir.ActivationFunctionType.Sigmoid
            )

            # fast-path k=0 weights for the first 128-row block, fed from the
            # tiny xi0 load so the first matmuls start early
            wk0_hi = const.tile([P, P], BF16, tag="wk0_hi")
            nc.vector.tensor_scalar_mul(wk0_hi[:], xi0[:], rel[:, 0:1])
            wk0b1 = const.tile([P, P], BF16, tag="wk0b1")
            nc.vector.tensor_scalar_mul(wk0b1[:], xic1[:], rel[:, 0:1])

            wks = {}

            def prep_wk(k):
                if k >= K or k in wks:
                    return
                t = wpool.tile([P, SHARD], BF16, tag="wk_hi")
                nc.vector.tensor_scalar_mul(t[:], xiT[:], rel[:, k : k + 1])
                wks[k] = t

            def dve_sigmoid(ps, obslice):
                """Clamped deg-13 odd-poly sigmoid on the DVE; first op
                copies the scores out of PSUM so the bank frees early."""
                xs = vpool.tile([P, HALF], F32, tag="xs")
                nc.vector.tensor_copy(xs[:], ps[:])
                w = vpool.tile([P, HALF], F32, tag="w")
                nc.vector.scalar_tensor_tensor(
                    w[:], xs[:], SIG_LAM, xs[:], ALU.mult, ALU.mult
                )
                ha = vpool.tile([P, HALF], F32, tag="ha")
                hb = vpool.tile([P, HALF], F32, tag="hb")
                cur, nxt = w, ha
                for s in SIG_S:
                    nc.vector.scalar_tensor_tensor(
                        nxt[:], cur[:], float(s), w[:], ALU.add, ALU.mult
                    )
                    cur, nxt = nxt, (hb if nxt is ha else ha)
                # t = (h + c0) * x ; sc = min(t + 0.5, 1) ; out = max(sc, 0)
                nc.vector.scalar_tensor_tensor(
                    nxt[:], cur[:], SIG_C0, xs[:], ALU.add, ALU.mult
                )
                sc = ha if nxt is hb else hb
                nc.vector.tensor_scalar(
                    sc[:], nxt[:], 0.5, 1.0, ALU.add, ALU.min
                )
                nc.vector.tensor_scalar_max(obslice, sc[:], 0.0)

            chunk = 0
            tix = 0  # regular-tile counter
            for k in range(K):
                prep_wk(k)
                for m in range(SHARD // P):  # 4 row blocks of 128
                    mc = slice(m * P, (m + 1) * P)
                    if k == 0 and m == 0:
                        # extra-fine first block: a leading 512-wide sub-chunk
                        # fed from the tiny duplicated loads so the store
                        # stream starts while the PE is still ramping
                        subs = [
                            (0, 512, xjh0a, 0),
                            (512, 512, rh[0], 512),
                            (1024, 1024, rh[1], 0),
                            (2048, 1024, rh[2], 0),
                            (3072, 1024, rh[3], 0),
                        ]
                        for c0, w_, th, off in subs:
                            psq = psum.tile([P, w_], F32, tag="ps")
                            for n2 in range(w_ // 512):
                                nc.tensor.matmul(
                                    psq[:, n2 * 512 : (n2 + 1) * 512],
                                    wk0_hi[:],
                                    th[:, off + n2 * 512 : off + (n2 + 1) * 512],
                                    start=True,
                                    stop=True,
                                )
                            obq = obuf_small.tile([P, w_], F16, tag="obs")
                            nc.scalar.activation(
                                obq[:], psq[:], mybir.ActivationFunctionType.Sigmoid
                            )
                            eng = nc.sync if chunk % 2 == 0 else nc.gpsimd
                            eng.dma_start(out[0, 0:P, c0 : c0 + w_], obq[:])
                            chunk += 1
                        continue
                    if m == 1:
                        prep_wk(k + 1)
                    wk_slice = wk0b1[:] if (k == 0 and m == 1) else wks[k][:, mc]
                    fine = k == K - 1 and m == SHARD // P - 1
                    ob = None if fine else obuf.tile([P, N_J], F16, tag="ob")
                    for h in range(2):  # two 2048-wide PSUM tiles per block
                        ps = psum.tile([P, HALF], F32, tag="ps")
                        for n4 in range(4):  # one 512-wide matmul per bank
                            gc = h * HALF + n4 * 512
                            nc.tensor.matmul(
                                ps[:, n4 * 512 : (n4 + 1) * 512],
                                wk_slice,
                                rh[gc // 1024][:, gc % 1024 : gc % 1024 + 512],
                                start=True,
                                stop=True,
                            )
                        if fine:
                            if h == 0:
                                obh = obuf_small.tile([P, HALF], F16, tag="obs")
                                nc.scalar.activation(
                                    obh[:], ps[:],
                                    mybir.ActivationFunctionType.Sigmoid,
                                )
                                nc.sync.dma_start(out[k, mc, 0:HALF], obh[:])
                            else:
                                # taper the very last stores (1024+512+512) so
                                # the kernel-final DMA is small before drain
                                for o0, w_, eng in (
                                    (0, 1024, nc.gpsimd),
                                    (1024, 512, nc.scalar),
                                    (1536, 512, nc.sync),
                                ):
                                    obt = obuf_small.tile([P, w_], F16, tag="obs")
                                    nc.scalar.activation(
                                        obt[:], ps[:, o0 : o0 + w_],
                                        mybir.ActivationFunctionType.Sigmoid,
                                    )
                                    eng.dma_start(
                                        out[k, mc, HALF + o0 : HALF + o0 + w_],
                                        obt[:],
                                    )
                            chunk += 1
                        else:
                            obslice = ob[:, h * HALF : (h + 1) * HALF]
                            if mode == "hv" and tix in DVE_TILES:
                                dve_sigmoid(ps, obslice)
                            else:
                                nc.scalar.activation(
                                    obslice, ps[:],
                                    mybir.ActivationFunctionType.Sigmoid,
                                )
                            tix += 1
                    if not fine:
                        if k == K - 1 and m >= 1:
                            # tail blocks: split across both rings so the
                            # store backlog drains before the kernel-end
                            nc.sync.dma_start(out[k, mc, 0:HALF], ob[:, 0:HALF])
                            nc.gpsimd.dma_start(out[k, mc, HALF:], ob[:, HALF:])
                        else:
                            eng = nc.sync if chunk % 2 == 0 else nc.gpsimd
                            eng.dma_start(out[k, mc, :], ob[:])
                        chunk += 1

    _split_ctrl_waits(nc)
    return nc


_cache = {}


def kernel(x_i, x_j, relations):
    x_i = np.asarray(x_i, dtype=np.float32)
    x_j = np.asarray(x_j, dtype=np.float32)
    relations = np.asarray(relations, dtype=np.float32)
    assert x_i.shape == (N_I, D) and x_j.shape == (N_J, D)
    assert relations.shape == (K, D)

    if MODE not in _cache:
        _cache[MODE] = build(MODE)
    nc = _cache[MODE]

    import ml_dtypes

    x_jT = np.ascontiguousarray(x_j.T)
    relT = np.ascontiguousarray(relations.T)
    common = {"relT": relT, "x_jT_hi": x_jT.astype(ml_dtypes.bfloat16)}

    in_maps = []
    for c in range(N_CORES):
        shard = np.ascontiguousarray(x_i[c * SHARD : (c + 1) * SHARD, :].T)
        in_maps.append(
            {"x_iT": shard, "x_i0T": np.ascontiguousarray(shard[:, 0:P]), **common}
        )

    trace = bool(int(os.environ.get("DISTMULT_TRACE", "0")))
    res = run_bass_kernel_spmd(nc, in_maps, list(range(N_CORES)), trace=trace)
    if trace:
        kernel.last_exec_time_ns = res.exec_time_ns
        kernel.last_results = res
    halves = [res.results[c]["out"] for c in range(N_CORES)]
    return np.concatenate(halves, axis=1).astype(np.float32)
